# revision 36
# baseline (speedup 1.0000x reference)
"""Neighbour3dAttnProcessor Trainium2 kernel.

3D neighborhood attention (NATTEN, window 7x7x7 over T=16,H=32,W=32, 8 heads,
hd=64) + QKV/output projections, sharded over 8 NeuronCores by the H axis:
core i owns query rows h in [4i, 4i+4) and receives a 10-row K/V halo slice
(zero-padded at the borders; padding is excluded by the attention mask).

Attention-mask trick: scores are computed as K_aug^T @ Q_aug where rows 64..103
of the fp16 contraction carry one-hot key-position features (value NEG) on the
K side and {0,1} window-violation indicators on the Q side - the matmul then
produces raw_score - |NEG| * (#violated window axes).  Row 100 is a constant
bias adding +SHIFT to every score so the vector/pool-engine exp needs no add.

Softmax exp is split across three engines:
  - ScalarE: exact exp via activation(Exp, bias=-SHIFT) on the leading chunks.
  - VectorE + Pool: Schraudolph bit-trick - i16 = max(score_shifted * L, 0)
    truncated to int16 IS the bf16 bit pattern of exp(score) (L = 128*log2 e,
    SHIFT*L = the bf16 exponent offset 16251).  Masked scores clamp to +0.0.

Tokens are reordered host-side to (w, h, t) so each query block (a pair of
w-columns, nq=128) attends a contiguous run of 10-11 aligned 128-key chunks.
Scores land keys-on-partitions; the AV matmul uses them as the stationary
operand and streams V's 65 columns (64 dims + a ones column), yielding
[query, dim] tiles whose softmax denominator is a per-partition scalar - so
normalization is reciprocal + per-partition scale on the Pool engine.  The
[tok, c] -> [c, tok] layout flip for the output projection goes through the
DMA crossbar (dma_start_transpose, 32x32 xbar tiles) instead of the PE.

Scheduling: Q projection runs first, then the K/V projection loop; attention
block-pipelines are emitted inside the K/V loop as soon as their key chunks
are resident, so there is no phase barrier and the PE stays busy end to end.
"""

import numpy as np

import concourse.bass as bass
import concourse.tile as tile
from concourse import bacc, mybir
from concourse.bass_utils import run_bass_kernel_spmd

BF16 = mybir.dt.bfloat16
F16 = mybir.dt.float16
F32 = mybir.dt.float32
I16 = mybir.dt.int16

T, H, W = 16, 32, 32
KT = KS = 7
HEADS, HD = 8, 64
C = HEADS * HD  # 512

N_CORES = 8
RH = 4          # own h rows per core
KVH = 10        # halo h rows per core
NQ = W * RH * T       # 2048 query tokens per core, order (w, j, t)
NKV = W * KVH * T     # 5120 kv tokens per core, order (w, hl, t)
WCOL = KVH * T        # 160 kv tokens per w column

R_T, R_H, R_W = 16, 10, 10      # w one-hots stored mod 10 (block spans
                                # at most 10 w-cols incl chunk-alignment
                                # spill, so kw%10 is a bijection on them)
NAUG = R_T + R_H + R_W + 1      # 36 mask rows + 1 bias row = 37
NMR = 40                        # mask rows padded to 40 for the DMA
KUSE = HD + NMR                 # 104 contraction rows actually used
NEG = -256.0                    # mask penalty per violated axis
SHIFT = 88.0                    # score bias so schraudolph needs no add
SCH_L = 16251.0 / 88.0          # ~2^7*log2(e), tuned so SHIFT*L = 16251

NBLK = NQ // 128                # 16 query blocks (one per w-column pair)
NCHUNK = NKV // 128             # 40 key chunks
DVE_CH = 4                      # trailing scb chunks (schraudolph engines)
ACT_CH = 6                      # leading sca chunks on exact ScalarE exp
POOL_SCB = 2                    # scb chunks on Pool (rest on VectorE)


def _block_chunks(b):
    """Key chunk index range for query block b (w columns 2b, 2b+1)."""
    ws = min(max(2 * b - 3, 0), W - 8)
    lo = ws * WCOL
    hi = lo + 8 * WCOL
    return lo // 128, -(-hi // 128)


def build_nc():
    nc = bacc.Bacc(None, target_bir_lowering=False)

    # xTp[p, k, t] = x^T[128k+p, t]: channel k-tiles packed side by side so
    # one DMA per 512-token group feeds all four contraction tiles.
    xTp = nc.declare_dram_parameter("xTp", [128, 4 * NKV], F16, isOutput=False)
    xQp = nc.declare_dram_parameter("xQp", [128, 4 * NQ], F16, isOutput=False)
    wkP = nc.declare_dram_parameter("wkP", [128, 4 * C], F16, isOutput=False)
    wvP = nc.declare_dram_parameter("wvP", [128, 4 * C], F16, isOutput=False)
    wqP = nc.declare_dram_parameter("wqP", [128, 4 * C], F16, isOutput=False)
    woP = nc.declare_dram_parameter("woP", [128, 4 * C], F16, isOutput=False)
    Fm = nc.declare_dram_parameter("Fm", [NMR, NKV], F16, isOutput=False)
    Gm = nc.declare_dram_parameter("Gm", [NMR, NQ], F16, isOutput=False)
    idm = nc.declare_dram_parameter("idm", [128, 128], F16, isOutput=False)
    out = nc.declare_dram_parameter("out", [NQ, C], F16, isOutput=True)

    with tile.TileContext(nc) as tc:
        with (
            tc.tile_pool(name="persist", bufs=1) as pp,
            tc.tile_pool(name="stream", bufs=2) as sp,
            tc.tile_pool(name="psum", bufs=2, space="PSUM") as qq,
            tc.tile_pool(name="psum1", bufs=1, space="PSUM") as q1,
        ):
            # 7 rotating [128, 65] AV slots packed into a single PSUM bank
            av4 = q1.tile([128, 7, 65], F32, name="av4", tag="av4")
            ka = [pp.tile([128, NKV], F16, name=f"ka{h}", tag=f"ka{h}")
                  for h in range(HEADS)]
            qa = [pp.tile([128, NQ], F16, name=f"qa{h}", tag=f"qa{h}")
                  for h in range(HEADS)]
            vta = pp.tile([128, NCHUNK * 8 * 65], BF16, name="vta", tag="vta")
            vtv = vta.rearrange("p (c h d) -> p c h d", c=NCHUNK, h=8)
            woS = pp.tile([128, 4 * C], F16, name="woS", tag="woS")
            ident = pp.tile([128, 128], F16, name="ident", tag="ident")
            nbias = pp.tile([128, 1], F32, name="nbias", tag="nbias")

            # ---- preamble: fill constants, mask features, weights ----
            nc.gpsimd.memset(nbias[:, :], -SHIFT)
            # the 65th (softmax-denominator ones) column of every head slot
            nc.gpsimd.memset(vtv[:, :, :, 64:65], 1.0)
            # mask-feature DMAs spread over three queues so they all land
            # before the first attention block needs them
            # all big mask DMAs go on the Pool queue (otherwise idle in
            # phase 1); qa0-3 on ACT ahead of its first evacuation copies.
            for h in range(4):
                nc.scalar.dma_start(out=qa[h][HD:KUSE, :], in_=Gm[:, :])
            for h in range(5):
                nc.gpsimd.dma_start(out=ka[h][HD:KUSE, :], in_=Fm[:, :])
            for h in range(5, 8):
                nc.gpsimd.dma_start(out=qa[h - 1][HD:KUSE, :], in_=Gm[:, :])
                nc.gpsimd.dma_start(out=ka[h][HD:KUSE, :], in_=Fm[:, :])
            nc.gpsimd.dma_start(out=qa[7][HD:KUSE, :], in_=Gm[:, :])

            # ---- evacuation engine rotation; the mix is swapped per phase
            # (Pool only joins once its mask-DMA backlog has drained, DVE
            # cannot issue DMAs so it never head-of-line blocks) ----
            rot = [nc.vector, nc.scalar]
            rot_i = [0]

            def evac(dst, src):
                eng = rot[rot_i[0] % len(rot)]
                rot_i[0] += 1
                if eng is nc.scalar:
                    eng.copy(dst, src)
                else:
                    eng.tensor_copy(dst, src)

            # ---- attention step machinery ----
            nstep = [0]          # completed attention steps (for engine picks)
            pend = []            # deferred (b, h, ex, ot2b), depth 2
            done2 = {}           # b -> ot2b pending transpose
            done3 = {}           # b -> otT pending projection

            def emit_scores(b, h):
                c0, c1 = _block_chunks(b)
                nch = c1 - c0
                ach = nch - DVE_CH
                # sca is 8 chunks (same 2-bank rounding as 7): the first
                # schraudolph chunk rides in its double-buffered tail so the
                # single-buffered scb WAR window shrinks to 3 chunks.
                sca = qq.tile([128, 8 * 128], F32, name=f"sca{b}_{h}",
                              tag="sca", bufs=2)
                scb = qq.tile([128, (DVE_CH - 1) * 128], F32,
                              name=f"scb{b}_{h}", tag="scb", bufs=1)
                for ci in range(nch):
                    if ci <= ach:
                        dst = sca[:, 128 * ci:128 * ci + 128]
                    else:
                        j = ci - ach - 1
                        dst = scb[:, 128 * j:128 * j + 128]
                    nc.tensor.matmul(
                        dst,
                        ka[h][0:KUSE, 128 * (c0 + ci):128 * (c0 + ci) + 128],
                        qa[h][0:KUSE, 128 * b:128 * (b + 1)],
                        start=True, stop=True)
                return sca, scb, nch, ach

            def emit_exp(b, h, sca, scb, nch, ach):
                ex = sp.tile([128, 11 * 128], BF16, name=f"ex{b}_{h}",
                             tag="ex", bufs=3)
                nc.vector.tensor_scalar(
                    ex[:, 128 * (ach + 1):128 * nch].bitcast(I16),
                    scb[:, 0:128 * (nch - ach - 1)], SCH_L, 0.0,
                    mybir.AluOpType.mult, mybir.AluOpType.max)
                nc.vector.tensor_scalar(
                    ex[:, 128 * ach:128 * (ach + 1)].bitcast(I16),
                    sca[:, 128 * ach:128 * (ach + 1)], SCH_L, 0.0,
                    mybir.AluOpType.mult, mybir.AluOpType.max)
                nc.scalar.activation(
                    ex[:, 0:128 * ach], sca[:, 0:128 * ach],
                    mybir.ActivationFunctionType.Exp, bias=nbias[:, :])
                return ex

            def emit_av_norm(b, h, ex, ot2b):
                c0, c1 = _block_chunks(b)
                nch = c1 - c0
                av = av4[:, (8 * b + h) % 7, :]
                for ci in range(nch):
                    nc.tensor.matmul(
                        av,
                        ex[:, 128 * ci:128 * ci + 128],
                        vta[:, 520 * (c0 + ci) + 65 * h:
                            520 * (c0 + ci) + 65 * h + 65],
                        start=(ci == 0), stop=(ci == nch - 1))
                rec = sp.tile([128, 1], F32, name=f"rec{b}_{h}", tag="rec",
                              bufs=2)
                nc.vector.reciprocal(rec[:, :], av[:, 64:65])
                nc.vector.tensor_scalar(
                    ot2b[:, HD * h:HD * h + HD], av[:, 0:HD], rec[:, :],
                    None, mybir.AluOpType.mult)
                nstep[0] += 1

            def emit_transpose(b, ot2b):
                otT = sp.tile([128, 512], F16, name=f"otT{b}", tag="otT",
                              bufs=2)
                nc.sync.dma_start_transpose(
                    out=otT.rearrange("p (j t) -> p j t", j=4),
                    in_=ot2b[:, :])
                return otT

            def emit_outproj(b, otT):
                po = qq.tile([128, 512], F32, name=f"po{b}", tag="mm")
                for p in range(4):
                    nc.tensor.matmul(po[:, :],
                                     otT[:, 128 * p:128 * (p + 1)],
                                     woS[:, 512 * p:512 * (p + 1)],
                                     start=(p == 0), stop=(p == 3))
                ob = sp.tile([128, 512], F16, name=f"ob{b}", tag="ob")
                nc.vector.tensor_copy(ob[:, 0:256], po[:, 0:256])
                nc.scalar.copy(ob[:, 256:512], po[:, 256:512])
                nc.sync.dma_start(out=out[128 * b:128 * (b + 1), :], in_=ob)

            def att_steps():
                for b in range(NBLK):
                    ot2b = sp.tile([128, C], F16, name=f"ot2_{b}", tag="ot2",
                                   bufs=2)
                    for h in range(HEADS):
                        yield b, h, ot2b

            steps = att_steps()
            next_step = [next(steps)]

            def pump(avail, cap, act_ch):
                """Emit up to `cap` attention steps whose key chunks are
                already resident (chunk index < avail)."""
                emitted = 0
                while next_step[0] is not None and emitted < cap:
                    b, h, ot2b = next_step[0]
                    if _block_chunks(b)[1] > avail:
                        return
                    sca, scb, nch, ach = emit_scores(b, h)
                    ex = emit_exp(b, h, sca, scb, nch, ach)
                    if len(pend) >= 2:
                        p0 = pend.pop(0)
                        emit_av_norm(*p0)
                        if p0[1] == HEADS - 1:
                            done2[p0[0]] = p0[3]
                    if h == 3 and (b - 1) in done2:
                        done3[b - 1] = emit_transpose(b - 1, done2.pop(b - 1))
                    if h == 6 and (b - 2) in done3:
                        emit_outproj(b - 2, done3.pop(b - 2))
                    pend.append((b, h, ex, ot2b))
                    emitted += 1
                    try:
                        next_step[0] = next(steps)
                    except StopIteration:
                        next_step[0] = None

            # ---- projections ----
            # While the attention pump is not yet running, the idle "sca"
            # PSUM slots double the projection PSUM depth so the PE never
            # waits on an evacuation.
            pcnt = [0]
            use_extra = [True]

            def proj_psum(name):
                pcnt[0] += 1
                if use_extra[0] and pcnt[0] % 2 == 0:
                    t = qq.tile([128, 896], F32, name=name + "x", tag="sca",
                                bufs=2)
                    return t[:, 0:512]
                return qq.tile([128, 512], F32, name=name, tag="mm")

            with tc.tile_pool(name="w1", bufs=1) as wp:
                wqS = wp.tile([128, 4 * C], F16, name="wqS", tag="wqS")
                wkS = wp.tile([128, 4 * C], F16, name="wkS", tag="wkS")
                wvS = wp.tile([128, 4 * C], F16, name="wvS", tag="wvS")
                nc.sync.dma_start(out=wqS, in_=wqP[:, :])
                xqs = []
                for n in range(NQ // 512):
                    xq = wp.tile([128, 2048], F16, name=f"xq{n}", tag="xq",
                                 bufs=3)
                    nc.sync.dma_start(
                        out=xq,
                        in_=xQp.rearrange("p (k t) -> p k t", k=4)
                        [:, :, 512 * n:512 * (n + 1)])
                    xqs.append(xq)
                    if n == 1:
                        nc.sync.dma_start(out=wkS, in_=wkP[:, :])
                        nc.sync.dma_start(out=wvS, in_=wvP[:, :])

                # Q projection over own NQ tokens
                for n in range(NQ // 512):
                    xq = xqs[n]
                    for p in range(4):
                        ps = proj_psum(f"pq{n}_{p}")
                        for k in range(4):
                            nc.tensor.matmul(
                                ps[:, :],
                                wqS[:, 512 * k + 128 * p:512 * k + 128 * p + 128],
                                xq[:, 512 * k:512 * (k + 1)],
                                start=(k == 0), stop=(k == 3))
                        evac(qa[2 * p][0:HD, 512 * n:512 * (n + 1)],
                             ps[0:HD, :])
                        evac(qa[2 * p + 1][0:HD, 512 * n:512 * (n + 1)],
                             ps[HD:128, :])

                nc.sync.dma_start(out=woS, in_=woP[:, :])
                nc.sync.dma_start(out=ident, in_=idm[:, :])

                # K/V over all NKV tokens, attention pumped in as chunks land
                xss = {}
                xv = xTp.rearrange("p (k t) -> p k t", k=4)
                for n in range(3):
                    xss[n] = wp.tile([128, 2048], F16, name=f"xs{n}",
                                     tag="xs", bufs=3)
                    nc.sync.dma_start(out=xss[n],
                                      in_=xv[:, :, 512 * n:512 * (n + 1)])
                for n in range(NKV // 512):
                    if n == 2:
                        use_extra[0] = False
                    if n >= 3:
                        pump(4 * n, 12, 0)
                    if n + 3 < NKV // 512:
                        xss[n + 3] = wp.tile([128, 2048], F16,
                                             name=f"xs{n + 3}", tag="xs",
                                             bufs=3)
                        nc.sync.dma_start(
                            out=xss[n + 3],
                            in_=xv[:, :, 512 * (n + 3):512 * (n + 4)])
                    xs = xss.pop(n)
                    for p in range(4):  # K head pairs
                        ps = proj_psum(f"pk{n}_{p}")
                        for k in range(4):
                            nc.tensor.matmul(
                                ps[:, :],
                                wkS[:, 512 * k + 128 * p:512 * k + 128 * p + 128],
                                xs[:, 512 * k:512 * (k + 1)],
                                start=(k == 0), stop=(k == 3))
                        evac(ka[2 * p][0:HD, 512 * n:512 * (n + 1)],
                             ps[0:HD, :])
                        evac(ka[2 * p + 1][0:HD, 512 * n:512 * (n + 1)],
                             ps[HD:128, :])
                    for s in range(4):  # V for four 128-token chunks
                        pv = proj_psum(f"pv{n}_{s}")
                        for k in range(4):
                            nc.tensor.matmul(
                                pv[:, :],
                                xs[:, 512 * k + 128 * s:512 * k + 128 * s + 128],
                                wvS[:, 512 * k:512 * (k + 1)],
                                start=(k == 0), stop=(k == 3))
                        evac(vtv[:, 4 * n + s, :, 0:64],
                             pv.rearrange("p (h d) -> p h d", h=8))

            # ---- attention drain ----
            while next_step[0] is not None:
                pump(NCHUNK, 1000, 0)
            for p0 in pend:
                emit_av_norm(*p0)
                if p0[1] == HEADS - 1:
                    done2[p0[0]] = p0[3]
            pend.clear()
            for b in sorted(done3):
                emit_outproj(b, done3.pop(b))
            # last blocks: PE transpose avoids the DMA-crossbar latency on
            # the critical tail
            for b in sorted(done2):
                ot2b = done2.pop(b)
                otT = sp.tile([128, 512], F16, name=f"otTz{b}", tag="otT",
                              bufs=2)
                tp = qq.tile([128, 512], F16, name=f"tp{b}", tag="mm")
                for p in range(4):
                    nc.tensor.transpose(tp[:, 128 * p:128 * (p + 1)],
                                        ot2b[:, 128 * p:128 * (p + 1)],
                                        ident[:, :])
                for p in range(4):
                    src = tp[:, 128 * p:128 * (p + 1)]
                    dst = otT[:, 128 * p:128 * (p + 1)]
                    if p % 2 == 0:
                        nc.vector.tensor_copy(dst, src)
                    else:
                        nc.scalar.copy(dst, src)
                emit_outproj(b, otT)

    nc.compile()
    return nc


def _host_inputs(hidden_states, w_q, w_k, w_v, w_o):
    f16 = np.float16
    xg = np.asarray(hidden_states, np.float32).reshape(H, W, T, C)
    xp = np.pad(xg, ((3, 3), (0, 0), (0, 0), (0, 0)))  # [38, W, T, C]

    # F: one-hot key position features, value NEG; plus bias row = SHIFT
    kk = np.arange(NKV)
    kw, khl, kt = kk // WCOL, (kk // T) % KVH, kk % T
    Fm = np.zeros((NMR, NKV), np.float32)
    Fm[kt, kk] = NEG
    Fm[R_T + khl, kk] = NEG
    Fm[R_T + R_H + (kw % 10), kk] = NEG
    Fm[R_T + R_H + R_W, :] = SHIFT
    Fm = Fm.astype(f16)

    qq_ = np.arange(NQ)
    qw, qj, qt = qq_ // (RH * T), (qq_ // T) % RH, qq_ % T
    ts = np.clip(qt - 3, 0, T - KT)
    wss = np.clip(qw - 3, 0, W - KS)

    def pack(w):
        return np.ascontiguousarray(
            np.asarray(w, np.float32).astype(f16).reshape(4, 128, C)
            .transpose(1, 0, 2).reshape(128, 4 * C))

    wqb, wkb, wvb, wob = pack(w_q), pack(w_k), pack(w_v), pack(w_o)

    ins = []
    for i in range(N_CORES):
        # kv slice: global rows 4i-3 .. 4i+7 == padded rows 4i .. 4i+10
        xs = xp[4 * i:4 * i + KVH]                      # [10, W, T, C]
        xT = xs.transpose(3, 1, 0, 2).reshape(C, NKV)
        xTp = np.ascontiguousarray(
            xT.reshape(4, 128, NKV).transpose(1, 0, 2)
            .reshape(128, 4 * NKV)).astype(f16)
        xTq = xg[4 * i:4 * i + RH].transpose(3, 1, 0, 2).reshape(C, NQ)
        xQp = np.ascontiguousarray(
            xTq.reshape(4, 128, NQ).transpose(1, 0, 2)
            .reshape(128, 4 * NQ)).astype(f16)

        Gm = np.zeros((NMR, NQ), np.float32)
        it = np.arange(R_T)[:, None]
        Gm[0:R_T] = ~((it >= ts[None, :]) & (it < ts[None, :] + KT))
        hglob = 4 * i + qj
        hs_loc = np.clip(hglob - 3, 0, H - KS) - (4 * i - 3)
        ih = np.arange(R_H)[:, None]
        Gm[R_T:R_T + R_H] = ~((ih >= hs_loc[None, :]) &
                              (ih < hs_loc[None, :] + KS))
        # w one-hots stored mod 10: c(r) is the unique w-col in the block's
        # chunk-extended span [ws-1, ws+9) with c % 10 == r
        wb = np.clip(2 * (qw // 2) - 3, 0, W - 8)
        ir = np.arange(R_W)[:, None]
        cr = (wb[None, :] - 1) + ((ir - (wb[None, :] - 1)) % 10)
        Gm[R_T + R_H:R_T + R_H + R_W] = ~(
            (cr >= wss[None, :]) & (cr < wss[None, :] + KS) &
            (cr >= wb[None, :]) & (cr < wb[None, :] + 8))
        Gm[R_T + R_H + R_W, :] = 1.0
        Gm = Gm.astype(f16)

        ins.append({
            "xTp": xTp, "xQp": xQp,
            "wqP": wqb, "wkP": wkb, "wvP": wvb, "woP": wob,
            "Fm": Fm, "Gm": Gm,
            "idm": np.eye(128, dtype=f16),
        })
    return ins


_NC_CACHE = None


def kernel(hidden_states, w_q, w_k, w_v, w_o, b_o):
    global _NC_CACHE
    if _NC_CACHE is None:
        _NC_CACHE = build_nc()
    nc = _NC_CACHE
    ins = _host_inputs(hidden_states, w_q, w_k, w_v, w_o)
    res = run_bass_kernel_spmd(nc, ins, core_ids=list(range(N_CORES)))

    full = np.empty((H, W, T, C), np.float32)
    for i in range(N_CORES):
        o = np.asarray(res.results[i]["out"]).astype(np.float32)
        full[4 * i:4 * i + RH] = o.reshape(W, RH, T, C).transpose(1, 0, 2, 3)
    full = full.reshape(H * W, T, C) + np.asarray(b_o, np.float32)
    return full


# revision 37
# speedup vs baseline: 1.0033x; 1.0033x over previous
"""Neighbour3dAttnProcessor Trainium2 kernel.

3D neighborhood attention (NATTEN, window 7x7x7 over T=16,H=32,W=32, 8 heads,
hd=64) + QKV/output projections, sharded over 8 NeuronCores by the H axis:
core i owns query rows h in [4i, 4i+4) and receives a 10-row K/V halo slice
(zero-padded at the borders; padding is excluded by the attention mask).

Attention-mask trick: scores are computed as K_aug^T @ Q_aug where rows 64..103
of the fp16 contraction carry one-hot key-position features (value NEG) on the
K side and {0,1} window-violation indicators on the Q side - the matmul then
produces raw_score - |NEG| * (#violated window axes).  Row 100 is a constant
bias adding +SHIFT to every score so the vector/pool-engine exp needs no add.

Softmax exp is split across three engines:
  - ScalarE: exact exp via activation(Exp, bias=-SHIFT) on the leading chunks.
  - VectorE + Pool: Schraudolph bit-trick - i16 = max(score_shifted * L, 0)
    truncated to int16 IS the bf16 bit pattern of exp(score) (L = 128*log2 e,
    SHIFT*L = the bf16 exponent offset 16251).  Masked scores clamp to +0.0.

Tokens are reordered host-side to (w, h, t) so each query block (a pair of
w-columns, nq=128) attends a contiguous run of 10-11 aligned 128-key chunks.
Scores land keys-on-partitions; the AV matmul uses them as the stationary
operand and streams V's 65 columns (64 dims + a ones column), yielding
[query, dim] tiles whose softmax denominator is a per-partition scalar - so
normalization is reciprocal + per-partition scale on the Pool engine.  The
[tok, c] -> [c, tok] layout flip for the output projection goes through the
DMA crossbar (dma_start_transpose, 32x32 xbar tiles) instead of the PE.

Scheduling: Q projection runs first, then the K/V projection loop; attention
block-pipelines are emitted inside the K/V loop as soon as their key chunks
are resident, so there is no phase barrier and the PE stays busy end to end.
"""

import numpy as np

import concourse.bass as bass
import concourse.tile as tile
from concourse import bacc, mybir
from concourse.bass_utils import run_bass_kernel_spmd

BF16 = mybir.dt.bfloat16
F16 = mybir.dt.float16
F32 = mybir.dt.float32
I16 = mybir.dt.int16

T, H, W = 16, 32, 32
KT = KS = 7
HEADS, HD = 8, 64
C = HEADS * HD  # 512

N_CORES = 8
RH = 4          # own h rows per core
KVH = 10        # halo h rows per core
NQ = W * RH * T       # 2048 query tokens per core, order (w, j, t)
NKV = W * KVH * T     # 5120 kv tokens per core, order (w, hl, t)
WCOL = KVH * T        # 160 kv tokens per w column

R_T, R_H, R_W = 16, 10, 10      # w one-hots stored mod 10 (block spans
                                # at most 10 w-cols incl chunk-alignment
                                # spill, so kw%10 is a bijection on them)
NAUG = R_T + R_H + R_W + 1      # 36 mask rows + 1 bias row = 37
NMR = 40                        # mask rows padded to 40 for the DMA
KUSE = HD + NMR                 # 104 contraction rows actually used
NEG = -256.0                    # mask penalty per violated axis
SHIFT = 88.0                    # score bias so schraudolph needs no add
SCH_L = 16251.0 / 88.0          # ~2^7*log2(e), tuned so SHIFT*L = 16251

NBLK = NQ // 128                # 16 query blocks (one per w-column pair)
NCHUNK = NKV // 128             # 40 key chunks
DVE_CH = 4                      # trailing scb chunks (schraudolph engines)
ACT_CH = 6                      # leading sca chunks on exact ScalarE exp
POOL_SCB = 2                    # scb chunks on Pool (rest on VectorE)


def _block_chunks(b):
    """Key chunk index range for query block b (w columns 2b, 2b+1)."""
    ws = min(max(2 * b - 3, 0), W - 8)
    lo = ws * WCOL
    hi = lo + 8 * WCOL
    return lo // 128, -(-hi // 128)


def build_nc():
    nc = bacc.Bacc(None, target_bir_lowering=False)

    # xTp[p, k, t] = x^T[128k+p, t]: channel k-tiles packed side by side so
    # one DMA per 512-token group feeds all four contraction tiles.
    xTp = nc.declare_dram_parameter("xTp", [128, 4 * NKV], F16, isOutput=False)
    xQp = nc.declare_dram_parameter("xQp", [128, 4 * NQ], F16, isOutput=False)
    wkP = nc.declare_dram_parameter("wkP", [128, 4 * C], F16, isOutput=False)
    wvP = nc.declare_dram_parameter("wvP", [128, 4 * C], F16, isOutput=False)
    wqP = nc.declare_dram_parameter("wqP", [128, 4 * C], F16, isOutput=False)
    woP = nc.declare_dram_parameter("woP", [128, 4 * C], F16, isOutput=False)
    Fm = nc.declare_dram_parameter("Fm", [NMR, NKV], F16, isOutput=False)
    Gm = nc.declare_dram_parameter("Gm", [NMR, NQ], F16, isOutput=False)
    idm = nc.declare_dram_parameter("idm", [128, 128], F16, isOutput=False)
    out = nc.declare_dram_parameter("out", [NQ, C], F16, isOutput=True)

    with tile.TileContext(nc) as tc:
        with (
            tc.tile_pool(name="persist", bufs=1) as pp,
            tc.tile_pool(name="stream", bufs=2) as sp,
            tc.tile_pool(name="psum", bufs=2, space="PSUM") as qq,
            tc.tile_pool(name="psum1", bufs=1, space="PSUM") as q1,
        ):
            # 7 rotating [128, 65] AV slots packed into a single PSUM bank
            av4 = q1.tile([128, 7, 65], F32, name="av4", tag="av4")
            ka = [pp.tile([128, NKV], F16, name=f"ka{h}", tag=f"ka{h}")
                  for h in range(HEADS)]
            qa = [pp.tile([128, NQ], F16, name=f"qa{h}", tag=f"qa{h}")
                  for h in range(HEADS)]
            vta = pp.tile([128, NCHUNK * 8 * 65], BF16, name="vta", tag="vta")
            vtv = vta.rearrange("p (c h d) -> p c h d", c=NCHUNK, h=8)
            woS = pp.tile([128, 4 * C], F16, name="woS", tag="woS")
            ident = pp.tile([128, 128], F16, name="ident", tag="ident")
            nbias = pp.tile([128, 1], F32, name="nbias", tag="nbias")

            # ---- preamble: fill constants, mask features, weights ----
            nc.gpsimd.memset(nbias[:, :], -SHIFT)
            # the 65th (softmax-denominator ones) column of every head slot
            nc.gpsimd.memset(vtv[:, :, :, 64:65], 1.0)
            # mask-feature DMAs spread over three queues so they all land
            # before the first attention block needs them
            # all big mask DMAs go on the Pool queue (otherwise idle in
            # phase 1); qa0-3 on ACT ahead of its first evacuation copies.
            for h in range(4):
                nc.scalar.dma_start(out=qa[h][HD:KUSE, :], in_=Gm[:, :])
            for h in range(5):
                nc.gpsimd.dma_start(out=ka[h][HD:KUSE, :], in_=Fm[:, :])
            for h in range(5, 8):
                nc.gpsimd.dma_start(out=qa[h - 1][HD:KUSE, :], in_=Gm[:, :])
                nc.gpsimd.dma_start(out=ka[h][HD:KUSE, :], in_=Fm[:, :])
            nc.gpsimd.dma_start(out=qa[7][HD:KUSE, :], in_=Gm[:, :])

            # ---- evacuation engine rotation; the mix is swapped per phase
            # (Pool only joins once its mask-DMA backlog has drained, DVE
            # cannot issue DMAs so it never head-of-line blocks) ----
            rot = [nc.vector, nc.scalar]
            rot_i = [0]

            def evac(dst, src):
                eng = rot[rot_i[0] % len(rot)]
                rot_i[0] += 1
                if eng is nc.scalar:
                    eng.copy(dst, src)
                else:
                    eng.tensor_copy(dst, src)

            # ---- attention step machinery ----
            nstep = [0]          # completed attention steps (for engine picks)
            pend = []            # deferred (b, h, ex, ot2b), depth 2
            done2 = {}           # b -> ot2b pending transpose
            done3 = {}           # b -> otT pending projection

            def emit_scores(b, h):
                c0, c1 = _block_chunks(b)
                nch = c1 - c0
                ach = nch - DVE_CH
                sca = qq.tile([128, 7 * 128], F32, name=f"sca{b}_{h}",
                              tag="sca", bufs=2)
                scb = qq.tile([128, DVE_CH * 128], F32, name=f"scb{b}_{h}",
                              tag="scb", bufs=1)
                for ci in range(nch):
                    if ci < ach:
                        dst = sca[:, 128 * ci:128 * ci + 128]
                    else:
                        j = ci - ach
                        dst = scb[:, 128 * j:128 * j + 128]
                    nc.tensor.matmul(
                        dst,
                        ka[h][0:KUSE, 128 * (c0 + ci):128 * (c0 + ci) + 128],
                        qa[h][0:KUSE, 128 * b:128 * (b + 1)],
                        start=True, stop=True)
                return sca, scb, nch, ach

            def emit_exp(b, h, sca, scb, nch, ach):
                ex = sp.tile([128, 11 * 128], BF16, name=f"ex{b}_{h}",
                             tag="ex", bufs=3)
                nc.vector.tensor_scalar(
                    ex[:, 128 * ach:128 * nch].bitcast(I16),
                    scb[:, :], SCH_L, 0.0,
                    mybir.AluOpType.mult, mybir.AluOpType.max)
                nc.scalar.activation(
                    ex[:, 0:128 * ach], sca[:, 0:128 * ach],
                    mybir.ActivationFunctionType.Exp, bias=nbias[:, :])
                return ex

            def emit_av_norm(b, h, ex, ot2b):
                c0, c1 = _block_chunks(b)
                nch = c1 - c0
                av = av4[:, (8 * b + h) % 7, :]
                for ci in range(nch):
                    nc.tensor.matmul(
                        av,
                        ex[:, 128 * ci:128 * ci + 128],
                        vta[:, 520 * (c0 + ci) + 65 * h:
                            520 * (c0 + ci) + 65 * h + 65],
                        start=(ci == 0), stop=(ci == nch - 1))
                rec = sp.tile([128, 1], F32, name=f"rec{b}_{h}", tag="rec",
                              bufs=2)
                nc.vector.reciprocal(rec[:, :], av[:, 64:65])
                nc.vector.tensor_scalar(
                    ot2b[:, HD * h:HD * h + HD], av[:, 0:HD], rec[:, :],
                    None, mybir.AluOpType.mult)
                nstep[0] += 1

            def emit_transpose(b, ot2b):
                otT = sp.tile([128, 512], F16, name=f"otT{b}", tag="otT",
                              bufs=2)
                nc.sync.dma_start_transpose(
                    out=otT.rearrange("p (j t) -> p j t", j=4),
                    in_=ot2b[:, :])
                return otT

            def emit_outproj(b, otT):
                po = qq.tile([128, 512], F32, name=f"po{b}", tag="mm")
                for p in range(4):
                    nc.tensor.matmul(po[:, :],
                                     otT[:, 128 * p:128 * (p + 1)],
                                     woS[:, 512 * p:512 * (p + 1)],
                                     start=(p == 0), stop=(p == 3))
                ob = sp.tile([128, 512], F16, name=f"ob{b}", tag="ob")
                nc.vector.tensor_copy(ob[:, 0:256], po[:, 0:256])
                nc.scalar.copy(ob[:, 256:512], po[:, 256:512])
                nc.sync.dma_start(out=out[128 * b:128 * (b + 1), :], in_=ob)

            def att_steps():
                for b in range(NBLK):
                    ot2b = sp.tile([128, C], F16, name=f"ot2_{b}", tag="ot2",
                                   bufs=2)
                    for h in range(HEADS):
                        yield b, h, ot2b

            steps = att_steps()
            next_step = [next(steps)]

            def pump(avail, cap, act_ch):
                """Emit up to `cap` attention steps whose key chunks are
                already resident (chunk index < avail)."""
                emitted = 0
                while next_step[0] is not None and emitted < cap:
                    b, h, ot2b = next_step[0]
                    if _block_chunks(b)[1] > avail:
                        return
                    sca, scb, nch, ach = emit_scores(b, h)
                    ex = emit_exp(b, h, sca, scb, nch, ach)
                    if len(pend) >= 2:
                        p0 = pend.pop(0)
                        emit_av_norm(*p0)
                        if p0[1] == HEADS - 1:
                            done2[p0[0]] = p0[3]
                    if h == 3 and (b - 1) in done2:
                        done3[b - 1] = emit_transpose(b - 1, done2.pop(b - 1))
                    if h == 6 and (b - 2) in done3:
                        emit_outproj(b - 2, done3.pop(b - 2))
                    pend.append((b, h, ex, ot2b))
                    emitted += 1
                    try:
                        next_step[0] = next(steps)
                    except StopIteration:
                        next_step[0] = None

            # ---- projections ----
            # While the attention pump is not yet running, the idle "sca"
            # PSUM slots double the projection PSUM depth so the PE never
            # waits on an evacuation.
            pcnt = [0]
            use_extra = [True]

            def proj_psum(name):
                pcnt[0] += 1
                if use_extra[0] and pcnt[0] % 2 == 0:
                    t = qq.tile([128, 896], F32, name=name + "x", tag="sca",
                                bufs=2)
                    return t[:, 0:512]
                return qq.tile([128, 512], F32, name=name, tag="mm")

            with tc.tile_pool(name="w1", bufs=1) as wp:
                wqS = wp.tile([128, 4 * C], F16, name="wqS", tag="wqS")
                wkS = wp.tile([128, 4 * C], F16, name="wkS", tag="wkS")
                wvS = wp.tile([128, 4 * C], F16, name="wvS", tag="wvS")
                nc.sync.dma_start(out=wqS, in_=wqP[:, :])
                xqs = []
                for n in range(NQ // 512):
                    xq = wp.tile([128, 2048], F16, name=f"xq{n}", tag="xq",
                                 bufs=3)
                    nc.sync.dma_start(
                        out=xq,
                        in_=xQp.rearrange("p (k t) -> p k t", k=4)
                        [:, :, 512 * n:512 * (n + 1)])
                    xqs.append(xq)
                    if n == 1:
                        nc.sync.dma_start(out=wkS, in_=wkP[:, :])
                        nc.sync.dma_start(out=wvS, in_=wvP[:, :])

                # Q projection over own NQ tokens
                for n in range(NQ // 512):
                    xq = xqs[n]
                    for p in range(4):
                        ps = proj_psum(f"pq{n}_{p}")
                        for k in range(4):
                            nc.tensor.matmul(
                                ps[:, :],
                                wqS[:, 512 * k + 128 * p:512 * k + 128 * p + 128],
                                xq[:, 512 * k:512 * (k + 1)],
                                start=(k == 0), stop=(k == 3))
                        evac(qa[2 * p][0:HD, 512 * n:512 * (n + 1)],
                             ps[0:HD, :])
                        evac(qa[2 * p + 1][0:HD, 512 * n:512 * (n + 1)],
                             ps[HD:128, :])

                nc.sync.dma_start(out=woS, in_=woP[:, :])
                nc.sync.dma_start(out=ident, in_=idm[:, :])

                # K/V over all NKV tokens, attention pumped in as chunks land
                xss = {}
                xv = xTp.rearrange("p (k t) -> p k t", k=4)
                for n in range(3):
                    xss[n] = wp.tile([128, 2048], F16, name=f"xs{n}",
                                     tag="xs", bufs=3)
                    nc.sync.dma_start(out=xss[n],
                                      in_=xv[:, :, 512 * n:512 * (n + 1)])
                for n in range(NKV // 512):
                    if n == 2:
                        use_extra[0] = False
                    if n >= 3:
                        pump(4 * n, 12, 0)
                    if n + 3 < NKV // 512:
                        xss[n + 3] = wp.tile([128, 2048], F16,
                                             name=f"xs{n + 3}", tag="xs",
                                             bufs=3)
                        nc.sync.dma_start(
                            out=xss[n + 3],
                            in_=xv[:, :, 512 * (n + 3):512 * (n + 4)])
                    xs = xss.pop(n)
                    for p in range(4):  # K head pairs
                        ps = proj_psum(f"pk{n}_{p}")
                        for k in range(4):
                            nc.tensor.matmul(
                                ps[:, :],
                                wkS[:, 512 * k + 128 * p:512 * k + 128 * p + 128],
                                xs[:, 512 * k:512 * (k + 1)],
                                start=(k == 0), stop=(k == 3))
                        evac(ka[2 * p][0:HD, 512 * n:512 * (n + 1)],
                             ps[0:HD, :])
                        evac(ka[2 * p + 1][0:HD, 512 * n:512 * (n + 1)],
                             ps[HD:128, :])
                    for s in range(4):  # V for four 128-token chunks
                        pv = proj_psum(f"pv{n}_{s}")
                        for k in range(4):
                            nc.tensor.matmul(
                                pv[:, :],
                                xs[:, 512 * k + 128 * s:512 * k + 128 * s + 128],
                                wvS[:, 512 * k:512 * (k + 1)],
                                start=(k == 0), stop=(k == 3))
                        evac(vtv[:, 4 * n + s, :, 0:64],
                             pv.rearrange("p (h d) -> p h d", h=8))

            # ---- attention drain ----
            while next_step[0] is not None:
                pump(NCHUNK, 1000, 0)
            for p0 in pend:
                emit_av_norm(*p0)
                if p0[1] == HEADS - 1:
                    done2[p0[0]] = p0[3]
            pend.clear()
            for b in sorted(done3):
                emit_outproj(b, done3.pop(b))
            # last blocks: PE transpose avoids the DMA-crossbar latency on
            # the critical tail; transpose/evac/project per 128-col piece so
            # only the last piece trails the final norm
            for b in sorted(done2):
                ot2b = done2.pop(b)
                otT = sp.tile([128, 512], F16, name=f"otTz{b}", tag="otT",
                              bufs=2)
                tp = qq.tile([128, 512], F16, name=f"tp{b}", tag="mm")
                po = qq.tile([128, 512], F32, name=f"poz{b}", tag="mm")
                for p in range(4):
                    nc.tensor.transpose(tp[:, 128 * p:128 * (p + 1)],
                                        ot2b[:, 128 * p:128 * (p + 1)],
                                        ident[:, :])
                    src = tp[:, 128 * p:128 * (p + 1)]
                    dst = otT[:, 128 * p:128 * (p + 1)]
                    if p % 2 == 0:
                        nc.vector.tensor_copy(dst, src)
                    else:
                        nc.scalar.copy(dst, src)
                    nc.tensor.matmul(po[:, :],
                                     otT[:, 128 * p:128 * (p + 1)],
                                     woS[:, 512 * p:512 * (p + 1)],
                                     start=(p == 0), stop=(p == 3))
                ob = sp.tile([128, 512], F16, name=f"obz{b}", tag="ob")
                nc.vector.tensor_copy(ob[:, 0:256], po[:, 0:256])
                nc.scalar.copy(ob[:, 256:512], po[:, 256:512])
                nc.sync.dma_start(out=out[128 * b:128 * (b + 1), :], in_=ob)

    nc.compile()
    return nc


def _host_inputs(hidden_states, w_q, w_k, w_v, w_o):
    f16 = np.float16
    xg = np.asarray(hidden_states, np.float32).reshape(H, W, T, C)
    xp = np.pad(xg, ((3, 3), (0, 0), (0, 0), (0, 0)))  # [38, W, T, C]

    # F: one-hot key position features, value NEG; plus bias row = SHIFT
    kk = np.arange(NKV)
    kw, khl, kt = kk // WCOL, (kk // T) % KVH, kk % T
    Fm = np.zeros((NMR, NKV), np.float32)
    Fm[kt, kk] = NEG
    Fm[R_T + khl, kk] = NEG
    Fm[R_T + R_H + (kw % 10), kk] = NEG
    Fm[R_T + R_H + R_W, :] = SHIFT
    Fm = Fm.astype(f16)

    qq_ = np.arange(NQ)
    qw, qj, qt = qq_ // (RH * T), (qq_ // T) % RH, qq_ % T
    ts = np.clip(qt - 3, 0, T - KT)
    wss = np.clip(qw - 3, 0, W - KS)

    def pack(w):
        return np.ascontiguousarray(
            np.asarray(w, np.float32).astype(f16).reshape(4, 128, C)
            .transpose(1, 0, 2).reshape(128, 4 * C))

    wqb, wkb, wvb, wob = pack(w_q), pack(w_k), pack(w_v), pack(w_o)

    ins = []
    for i in range(N_CORES):
        # kv slice: global rows 4i-3 .. 4i+7 == padded rows 4i .. 4i+10
        xs = xp[4 * i:4 * i + KVH]                      # [10, W, T, C]
        xT = xs.transpose(3, 1, 0, 2).reshape(C, NKV)
        xTp = np.ascontiguousarray(
            xT.reshape(4, 128, NKV).transpose(1, 0, 2)
            .reshape(128, 4 * NKV)).astype(f16)
        xTq = xg[4 * i:4 * i + RH].transpose(3, 1, 0, 2).reshape(C, NQ)
        xQp = np.ascontiguousarray(
            xTq.reshape(4, 128, NQ).transpose(1, 0, 2)
            .reshape(128, 4 * NQ)).astype(f16)

        Gm = np.zeros((NMR, NQ), np.float32)
        it = np.arange(R_T)[:, None]
        Gm[0:R_T] = ~((it >= ts[None, :]) & (it < ts[None, :] + KT))
        hglob = 4 * i + qj
        hs_loc = np.clip(hglob - 3, 0, H - KS) - (4 * i - 3)
        ih = np.arange(R_H)[:, None]
        Gm[R_T:R_T + R_H] = ~((ih >= hs_loc[None, :]) &
                              (ih < hs_loc[None, :] + KS))
        # w one-hots stored mod 10: c(r) is the unique w-col in the block's
        # chunk-extended span [ws-1, ws+9) with c % 10 == r
        wb = np.clip(2 * (qw // 2) - 3, 0, W - 8)
        ir = np.arange(R_W)[:, None]
        cr = (wb[None, :] - 1) + ((ir - (wb[None, :] - 1)) % 10)
        Gm[R_T + R_H:R_T + R_H + R_W] = ~(
            (cr >= wss[None, :]) & (cr < wss[None, :] + KS) &
            (cr >= wb[None, :]) & (cr < wb[None, :] + 8))
        Gm[R_T + R_H + R_W, :] = 1.0
        Gm = Gm.astype(f16)

        ins.append({
            "xTp": xTp, "xQp": xQp,
            "wqP": wqb, "wkP": wkb, "wvP": wvb, "woP": wob,
            "Fm": Fm, "Gm": Gm,
            "idm": np.eye(128, dtype=f16),
        })
    return ins


_NC_CACHE = None


def kernel(hidden_states, w_q, w_k, w_v, w_o, b_o):
    global _NC_CACHE
    if _NC_CACHE is None:
        _NC_CACHE = build_nc()
    nc = _NC_CACHE
    ins = _host_inputs(hidden_states, w_q, w_k, w_v, w_o)
    res = run_bass_kernel_spmd(nc, ins, core_ids=list(range(N_CORES)))

    full = np.empty((H, W, T, C), np.float32)
    for i in range(N_CORES):
        o = np.asarray(res.results[i]["out"]).astype(np.float32)
        full[4 * i:4 * i + RH] = o.reshape(W, RH, T, C).transpose(1, 0, 2, 3)
    full = full.reshape(H * W, T, C) + np.asarray(b_o, np.float32)
    return full
